# revision 26
# baseline (speedup 1.0000x reference)
"""GCN message-passing kernel for 8 trn2 NeuronCores.  ~56us HW.

Math (per reference): h = relu(a @ (x @ W1) + b1); out = h @ W2 + b2
Shapes: x [8,4096,240], a [4096,4096], W1 [240,32], W2 [32,240].

Sharding: 2x4 grid. Core c -> batch group g=c//4 (4 batches), output-row
group j=c%4 (1024 rows).  Per-core traffic: x 3.9MB + a 4.2MB + out 2MB.

Precision (end-to-end rel err 1.52e-2 vs the 2e-2 gate, deterministic):
  - a shipped as centered fp8 e3m4 (a-0.5); the exact rank-1 correction
    0.5*sum_m h[m,:] is one DVE row-reduce of hT folded into the ReLU
    bias with b1 (contributes 7.8e-3).
  - x shipped as fp8 e3m4 (contributes 1.3e-2); W1/W2/h stay fp16 --
    the PE accepts mixed fp16(stationary) x fp8(moving) matmuls at full
    internal precision (verified exact vs numpy on quantized operands).
  - b1==0 / b2==0 are detected on host and their paths compiled out.

Measured machine facts this schedule is built around:
  - PE retires ~1 column per 0.43ns regardless of operand dtype; per-
    matmul overhead is hidden at >=256 moving columns.  Total PE work:
    phase1 64x512col 14us + 32 transposes 1.8us + phase2 128x256col
    14us + 16 head matmuls 3.4us = ~33us.
  - The DMA engines are power-throttled to ~50-60% for the first ~15us,
    so the early x stream runs ~230GB/s while the late a stream gets
    ~400GB/s.  Startup to first packet is ~6us; the end-of-kernel
    barrier ceremony is ~3-8us.  Both are framework-fixed.
  - GpSimd cannot touch PSUM; gpsimd/scalar DGE queues are slow for
    bulk DMA -- everything streams on the sync queue, in order.
  - Collectives rendezvous costs ~40us here (cores start staggered),
    so the hx-AllGather sharding variant loses despite 4x less x.

Schedule: sync-queue order = xa0, w1a, xb0, w1b, x np=1..3, [b1], a
chunk 0 (split in halves), w2k, [b2], a chunks 1-3, 8 output stores.
Phase 1 accumulates each 512-col block as an xa-pass then an xb-pass
(first matmul needs only xa0+w1a = 0.6MB); PSUM drains alternate
Scalar/DVE; the 8 PE-transposes of pair np are emitted after pair
np+1's matmuls so the PE never waits on a drain.  Phase 2+3 run four
256-row chunks: 32 fp16xfp8 matmuls -> ReLU+bias on ACT -> block-
diagonal W2 head; chunk c's head is emitted after chunk c+1's matmuls
(no ACT wait on the PE), head-2 rides inside chunk-3, and each chunk's
two fp16 stores issue as soon as its drains finish.
"""

import sys

if "/opt/trn_rl_repo" not in sys.path:
    sys.path.insert(0, "/opt/trn_rl_repo")

import numpy as np
import ml_dtypes

B, N, F, H, L = 8, 4096, 240, 32, 240
NB = 4        # batches per core
NRC = 1024    # output rows per core
TRACE = False

_cache = {}
last_exec_time_ns = None
last_profile_json = None


def _install_ntff_hook():
    import types

    import antenv

    if "antenv.axon_hooks" in sys.modules:
        return
    mod = types.ModuleType("antenv.axon_hooks")
    _state = {"hook": None}
    mod.set_axon_ntff_profile_hook = lambda h: _state.__setitem__("hook", h)
    mod.get_axon_ntff_profile_hook = lambda: _state["hook"]
    sys.modules["antenv.axon_hooks"] = mod
    antenv.axon_hooks = mod
    from trn_agent_boot.trn_boot import _ntff_profile_via_ctypes

    mod.set_axon_ntff_profile_hook(
        _ntff_profile_via_ctypes("/opt/axon/libaxon_pjrt.so")
    )


def _build(has_b1, has_b2):
    import concourse.bass as bass
    import concourse.tile as tile
    from concourse import bacc, mybir

    f32 = mybir.dt.float32
    f16 = mybir.dt.float16
    f8 = mybir.dt.float8e3
    ts, ds = bass.ts, bass.ds

    nc = bacc.Bacc("TRN2", target_bir_lowering=False, debug=False, num_devices=8)
    xt = nc.dram_tensor("xt", [4 * F, N], f8, kind="ExternalInput").ap()
    atp = nc.dram_tensor("atp", [512, 8192], f8, kind="ExternalInput").ap()
    w1p = nc.dram_tensor("w1p", [F, 512], f16, kind="ExternalInput").ap()
    w2k = nc.dram_tensor("w2k", [128, 960], f16, kind="ExternalInput").ap()
    if has_b1:
        b1s = nc.dram_tensor("b1s", [128, 1], f32, kind="ExternalInput").ap()
    if has_b2:
        b2r = nc.dram_tensor("b2r", [1, 960], f16, kind="ExternalInput").ap()
    outp = nc.dram_tensor("outp", [128, 8 * NB * L], f16,
                          kind="ExternalOutput").ap()

    relu = mybir.ActivationFunctionType.Relu
    copyf = mybir.ActivationFunctionType.Copy
    AX = mybir.AxisListType.X
    add = mybir.AluOpType.add
    mult = mybir.AluOpType.mult

    with tile.TileContext(nc) as tc:
        with tc.tile_pool(name="const", bufs=1) as cp:
            # declared now, DMA'd interleaved with the x stream below
            w1a = cp.tile([128, 512], f16)
            w1b = cp.tile([112, 512], f16)
            w2s = cp.tile([128, 960], f16)
            if has_b1:
                b1t = cp.tile([128, 1], f32)
            idt = cp.tile([128, 128], f16)
            nc.gpsimd.memset(idt[:], 1.0)
            nc.gpsimd.affine_select(
                out=idt[:], in_=idt[:], compare_op=mybir.AluOpType.is_equal,
                fill=0.0, base=0, pattern=[[-1, 128]], channel_multiplier=1)
            if has_b2:
                b2t = cp.tile([1, 960], f16)
                ones = cp.tile([1, 128], f16)
                nc.vector.memset(ones[:], 1.0)
            hT = cp.tile([128, N], f16)
            hsb = cp.tile([128, N], f16)
            at4 = [cp.tile([128, 8192], f8, name=f"at_{k}") for k in range(4)]
            csum = cp.tile([128, 1], f32)
            bc = cp.tile([128, 1], f32)

            # phase 1: hT[32b+h, n] = sum_f W1[f,h] * x[b,n,f], n-block-major
            with tc.tile_pool(name="xs", bufs=1) as xs, \
                 tc.tile_pool(name="ps1", bufs=4, space="PSUM") as ps1, \
                 tc.tile_pool(name="pst", bufs=4, space="PSUM") as pst:
                xts = []
                for np_ in range(4):
                    xa = xs.tile([128, 4096], f8, name=f"xa_{np_}")
                    nc.sync.dma_start(xa[:], xt[ds(np_ * F, 128), :])
                    if np_ == 0:
                        # w1a right behind xa0: the first (xa-pass) matmul
                        # needs only these 0.625MB under the DMA throttle
                        nc.sync.dma_start(w1a[:], w1p[0:128, :])
                    xb = xs.tile([112, 4096], f8, name=f"xb_{np_}")
                    nc.sync.dma_start(xb[:], xt[ds(np_ * F + 128, 112), :])
                    xts.append((xa, xb))
                    if np_ == 0:
                        nc.sync.dma_start(w1b[:], w1p[128:240, :])

                def emit_transposes(np_):
                    for nn in range(2):
                        for q in range(4):
                            m = (2 * np_ + nn) * 4 + q
                            pt = pst.tile([128, 128], f16)
                            nc.tensor.transpose(pt[:], hT[:, ts(m, 128)],
                                                idt[:])
                            if q % 2 == 0:
                                nc.vector.tensor_copy(hsb[:, ts(m, 128)],
                                                      pt[:])
                            else:
                                nc.scalar.activation(hsb[:, ts(m, 128)],
                                                     pt[:], copyf)

                for np_ in range(4):
                    xa, xb = xts[np_]
                    for nn in range(2):
                        ncol = 2 * np_ + nn
                        p1 = ps1.tile([128, 512], f32)
                        for b in range(NB):
                            nc.tensor.matmul(p1[:], w1a[:, ts(b, 128)],
                                             xa[:, ds(b * 1024 + nn * 512,
                                                      512)],
                                             start=(b == 0), stop=False)
                        for b in range(NB):
                            nc.tensor.matmul(p1[:], w1b[:, ts(b, 128)],
                                             xb[:, ds(b * 1024 + nn * 512,
                                                      512)],
                                             start=False, stop=(b == NB - 1))
                        if nn == 0:
                            nc.scalar.activation(hT[:, ts(ncol, 512)], p1[:],
                                                 copyf)
                        else:
                            nc.vector.tensor_copy(hT[:, ts(ncol, 512)], p1[:])
                    if np_ > 0:
                        emit_transposes(np_ - 1)
                emit_transposes(3)

            # stream centered-fp8 a tiles; late consts slot in just before
            # their first use (b1t for the bias, w2s for chunk-0's head)
            if has_b1:
                nc.sync.dma_start(b1t[:], b1s[:])
            # first a tile in halves so chunk-0 starts ~1.5us earlier
            nc.sync.dma_start(at4[0][:, ds(0, 4096)],
                              atp[ds(0, 128), ds(0, 4096)])
            nc.sync.dma_start(at4[0][:, ds(4096, 4096)],
                              atp[ds(0, 128), ds(4096, 4096)])
            nc.sync.dma_start(w2s[:], w2k[:])
            if has_b2:
                nc.sync.dma_start(b2t[:], b2r[:])
            for k in range(1, 4):
                nc.sync.dma_start(at4[k][:], atp[ds(k * 128, 128), :])

            # bias = 0.5 * rowsum(hT) + b1  (rank-1 centering correction)
            nc.vector.tensor_reduce(csum[:], hT[:], axis=AX, op=add)
            if has_b1:
                nc.vector.tensor_scalar(bc[:], csum[:], 0.5, b1t[:],
                                        op0=mult, op1=add)
            else:
                nc.vector.tensor_scalar_mul(bc[:], csum[:], 0.5)

            # phase 2+3, four chunks of 256 output rows each. Chunk c's
            # W2 head is emitted after chunk c+1's kt-matmuls so the PE
            # never waits on the ReLU ACT at a chunk boundary.
            with tc.tile_pool(name="rs", bufs=3) as rs, \
                 tc.tile_pool(name="os", bufs=4) as osb, \
                 tc.tile_pool(name="ps2", bufs=2, space="PSUM") as ps2, \
                 tc.tile_pool(name="ps3", bufs=4, space="PSUM") as ps3:
                drains = [nc.vector, nc.scalar]
                state = {"dri": 0}
                ract = [None] * 4

                def emit_head(c):
                    # w2s[32b+h, hf*480 + b*120 + li] = W2[h, hf*120 + li]
                    r = ract[c]
                    for ss in range(2):
                        o = osb.tile([128, 960], f16)
                        for hf in range(2):
                            p3 = ps3.tile([128, 480], f32)
                            if has_b2:
                                nc.tensor.matmul(
                                    p3[:], ones[:], b2t[:, ts(hf, 480)],
                                    start=True, stop=False)
                            nc.tensor.matmul(
                                p3[:], r[:, ts(ss, 128)],
                                w2s[:, ts(hf, 480)],
                                start=(not has_b2), stop=True)
                            dst = o[:, ds(hf * 480, 480)]
                            eng = drains[state["dri"] % 2]
                            state["dri"] += 1
                            if eng is nc.scalar:
                                nc.scalar.activation(dst, p3[:], copyf)
                            else:
                                eng.tensor_copy(dst, p3[:])
                        nc.sync.dma_start(outp[:, ts(c * 2 + ss, 960)], o[:])

                for c in range(4):
                    pa = ps2.tile([128, 256], f32)
                    for kt in range(32):
                        nc.tensor.matmul(
                            pa[:], hsb[:, ts(kt, 128)],
                            at4[c][:, ds(kt * 256, 256)],
                            start=(kt == 0), stop=(kt == 31))
                        if c == 3 and kt == 15:
                            emit_head(2)
                    r = rs.tile([128, 256], f16)
                    nc.scalar.activation(r[:], pa[:], relu, bias=bc[:])
                    ract[c] = r
                    if 1 <= c <= 2:
                        emit_head(c - 1)
                emit_head(3)

    nc.compile()
    return nc


def kernel(x, a, W1, b1, W2, b2):
    global last_exec_time_ns, last_profile_json
    from concourse.bass_utils import run_bass_kernel_spmd

    x = np.asarray(x, np.float32)
    a = np.asarray(a, np.float32)
    W1 = np.asarray(W1, np.float32)
    b1 = np.asarray(b1, np.float32)
    W2 = np.asarray(W2, np.float32)
    b2 = np.asarray(b2, np.float32)

    has_b1 = bool(np.any(b1))
    has_b2 = bool(np.any(b2))
    key = ("nc", has_b1, has_b2)
    if key not in _cache:
        _cache[key] = _build(has_b1, has_b2)
    nc = _cache[key]

    # xt[g]: [960, 4096] f16; xt[np*240+f, b*1024+nn*512+q] =
    #   x[4g+b, (2np+nn)*512+q, f]
    xg = []
    for g in range(2):
        arr = x[g * NB:(g + 1) * NB]            # [4, 4096, 240]
        arr = arr.transpose(2, 0, 1)            # [f, b, n]
        arr = arr.reshape(F, NB, 4, 2, 512)     # [f, b, np, nn, q]
        arr = arr.transpose(2, 0, 1, 3, 4)      # [np, f, b, nn, q]
        xg.append(np.ascontiguousarray(
            arr.reshape(4 * F, N)).astype(ml_dtypes.float8_e3m4))

    # atp[j]: [512, 8192] f8e3 chunk-major; atp[c*128+p, kt*256+q] =
    #   e3m4(a[j*1024 + c*256 + q, kt*128+p] - 0.5)
    a8 = (a.T - np.float32(0.5)).astype(ml_dtypes.float8_e3m4)  # [m, n_out]
    aj = []
    for j in range(4):
        Aj = a8[:, j * NRC:(j + 1) * NRC]       # [4096 m, 1024 n]
        arr = Aj.reshape(32, 128, 4, 256)       # [kt, p, c, q]
        arr = arr.transpose(2, 1, 0, 3)         # [c, p, kt, q]
        aj.append(np.ascontiguousarray(arr.reshape(512, 8192)))

    w1p = np.zeros((F, 512), np.float16)
    for b in range(NB):
        w1p[:, 128 * b + 32 * b:128 * b + 32 * b + 32] = W1.astype(np.float16)
    # w2k[32b+h, hf*480 + b*120 + li] = W2[h, hf*120 + li]; zeros elsewhere
    w2k = np.zeros((128, 960), np.float16)
    for hf in range(2):
        for b in range(NB):
            w2k[32 * b:32 * b + 32, 480 * hf + 120 * b:480 * hf + 120 * b + 120] = \
                W2[:, 120 * hf:120 * hf + 120].astype(np.float16)
    b1s = np.ascontiguousarray(np.tile(b1, 4).reshape(128, 1))

    ins = []
    for c in range(8):
        g, j = c // 4, c % 4
        d = {"xt": xg[g], "atp": aj[j], "w1p": w1p, "w2k": w2k}
        if has_b1:
            d["b1s"] = b1s
        if has_b2:
            b2r = np.empty((1, 960), np.float16)
            for hf in range(2):
                for b in range(NB):
                    b2r[0, 480 * hf + 120 * b:480 * hf + 120 * b + 120] = \
                        b2[120 * hf:120 * hf + 120].astype(np.float16)
            d["b2r"] = b2r
        ins.append(d)

    trace = TRACE
    if trace:
        try:
            _install_ntff_hook()
        except Exception:
            trace = False
    r = run_bass_kernel_spmd(nc, ins, list(range(8)), trace=trace)
    last_exec_time_ns = r.exec_time_ns
    last_profile_json = r.profile_json

    res = np.empty((B, N, L), np.float32)
    for c in range(8):
        g, j = c // 4, c % 4
        # outp[p, S, hf, b, li]; n = S*128 + p; l = hf*120+li
        arr = r.results[c]["outp"].reshape(128, 8, 2, NB, 120)
        res[g * NB:(g + 1) * NB, j * NRC:(j + 1) * NRC, :] = \
            arr.transpose(3, 1, 0, 2, 4).reshape(NB, NRC, L).astype(np.float32)
    return res


# revision 27
# speedup vs baseline: 1.0116x; 1.0116x over previous
"""GCN message-passing kernel for 8 trn2 NeuronCores.  ~56us HW.

Math (per reference): h = relu(a @ (x @ W1) + b1); out = h @ W2 + b2
Shapes: x [8,4096,240], a [4096,4096], W1 [240,32], W2 [32,240].

Sharding: 2x4 grid. Core c -> batch group g=c//4 (4 batches), output-row
group j=c%4 (1024 rows).  Per-core traffic: x 3.9MB + a 4.2MB + out 2MB.

Precision (end-to-end rel err 1.52e-2 vs the 2e-2 gate, deterministic):
  - a shipped as centered fp8 e3m4 (a-0.5); the exact rank-1 correction
    0.5*sum_m h[m,:] is one DVE row-reduce of hT folded into the ReLU
    bias with b1 (contributes 7.8e-3).
  - x shipped as fp8 e3m4 (contributes 1.3e-2); W1/W2/h stay fp16 --
    the PE accepts mixed fp16(stationary) x fp8(moving) matmuls at full
    internal precision (verified exact vs numpy on quantized operands).
  - b1==0 / b2==0 are detected on host and their paths compiled out.

Measured machine facts this schedule is built around:
  - PE retires ~1 column per 0.43ns regardless of operand dtype; per-
    matmul overhead is hidden at >=256 moving columns.  Total PE work:
    phase1 64x512col 14us + 32 transposes 1.8us + phase2 128x256col
    14us + 16 head matmuls 3.4us = ~33us.
  - The DMA engines are power-throttled to ~50-60% for the first ~15us,
    so the early x stream runs ~230GB/s while the late a stream gets
    ~400GB/s.  Startup to first packet is ~6us; the end-of-kernel
    barrier ceremony is ~3-8us.  Both are framework-fixed.
  - GpSimd cannot touch PSUM; gpsimd/scalar DGE queues are slow for
    bulk DMA -- everything streams on the sync queue, in order.
  - Collectives rendezvous costs ~40us here (cores start staggered),
    so the hx-AllGather sharding variant loses despite 4x less x.

Schedule: sync-queue order = xa0, w1a, xb0, w1b, x np=1..3, [b1], a
chunk 0 (split in halves), w2k, [b2], a chunks 1-3, 8 output stores.
Phase 1 accumulates each 512-col block as an xa-pass then an xb-pass
(first matmul needs only xa0+w1a = 0.6MB); PSUM drains alternate
Scalar/DVE; the 8 PE-transposes of pair np are emitted after pair
np+1's matmuls so the PE never waits on a drain.  Phase 2+3 run four
256-row chunks: 32 fp16xfp8 matmuls -> ReLU+bias on ACT -> block-
diagonal W2 head; chunk c's head is emitted after chunk c+1's matmuls
(no ACT wait on the PE), head-2 rides inside chunk-3, and each chunk's
two fp16 stores issue as soon as its drains finish.
"""

import sys

if "/opt/trn_rl_repo" not in sys.path:
    sys.path.insert(0, "/opt/trn_rl_repo")

import numpy as np
import ml_dtypes

B, N, F, H, L = 8, 4096, 240, 32, 240
NB = 4        # batches per core
NRC = 1024    # output rows per core
TRACE = False

_cache = {}
last_exec_time_ns = None
last_profile_json = None


def _install_ntff_hook():
    import types

    import antenv

    if "antenv.axon_hooks" in sys.modules:
        return
    mod = types.ModuleType("antenv.axon_hooks")
    _state = {"hook": None}
    mod.set_axon_ntff_profile_hook = lambda h: _state.__setitem__("hook", h)
    mod.get_axon_ntff_profile_hook = lambda: _state["hook"]
    sys.modules["antenv.axon_hooks"] = mod
    antenv.axon_hooks = mod
    from trn_agent_boot.trn_boot import _ntff_profile_via_ctypes

    mod.set_axon_ntff_profile_hook(
        _ntff_profile_via_ctypes("/opt/axon/libaxon_pjrt.so")
    )


def _build(has_b1, has_b2):
    import concourse.bass as bass
    import concourse.tile as tile
    from concourse import bacc, mybir

    f32 = mybir.dt.float32
    f16 = mybir.dt.float16
    f8 = mybir.dt.float8e3
    ts, ds = bass.ts, bass.ds

    nc = bacc.Bacc("TRN2", target_bir_lowering=False, debug=False, num_devices=8)
    xt = nc.dram_tensor("xt", [4 * F, N], f8, kind="ExternalInput").ap()
    atp = nc.dram_tensor("atp", [512, 8192], f8, kind="ExternalInput").ap()
    w1p = nc.dram_tensor("w1p", [F, 512], f16, kind="ExternalInput").ap()
    w2k = nc.dram_tensor("w2k", [128, 960], f16, kind="ExternalInput").ap()
    if has_b1:
        b1s = nc.dram_tensor("b1s", [128, 1], f32, kind="ExternalInput").ap()
    if has_b2:
        b2r = nc.dram_tensor("b2r", [1, 960], f16, kind="ExternalInput").ap()
    outp = nc.dram_tensor("outp", [128, 8 * NB * L], f16,
                          kind="ExternalOutput").ap()

    relu = mybir.ActivationFunctionType.Relu
    copyf = mybir.ActivationFunctionType.Copy
    AX = mybir.AxisListType.X
    add = mybir.AluOpType.add
    mult = mybir.AluOpType.mult

    with tile.TileContext(nc) as tc:
        with tc.tile_pool(name="const", bufs=1) as cp:
            # declared now, DMA'd interleaved with the x stream below
            w1a = cp.tile([128, 512], f16)
            w1b = cp.tile([112, 512], f16)
            w2s = cp.tile([128, 960], f16)
            if has_b1:
                b1t = cp.tile([128, 1], f32)
            idt = cp.tile([128, 128], f16)
            nc.gpsimd.memset(idt[:], 1.0)
            nc.gpsimd.affine_select(
                out=idt[:], in_=idt[:], compare_op=mybir.AluOpType.is_equal,
                fill=0.0, base=0, pattern=[[-1, 128]], channel_multiplier=1)
            if has_b2:
                b2t = cp.tile([1, 960], f16)
                ones = cp.tile([1, 128], f16)
                nc.vector.memset(ones[:], 1.0)
            hT = cp.tile([128, N], f16)
            hsb = cp.tile([128, N], f16)
            at4 = [cp.tile([128, 8192], f8, name=f"at_{k}") for k in range(4)]
            csum = cp.tile([128, 1], f32)
            bc = cp.tile([128, 1], f32)

            # phase 1: hT[32b+h, n] = sum_f W1[f,h] * x[b,n,f], n-block-major
            with tc.tile_pool(name="xs", bufs=1) as xs, \
                 tc.tile_pool(name="ps1", bufs=4, space="PSUM") as ps1, \
                 tc.tile_pool(name="pst", bufs=4, space="PSUM") as pst:
                xts = []
                for np_ in range(4):
                    xa = xs.tile([128, 4096], f8, name=f"xa_{np_}")
                    nc.sync.dma_start(xa[:], xt[ds(np_ * F, 128), :])
                    if np_ == 0:
                        # w1a right behind xa0: the first (xa-pass) matmul
                        # needs only these 0.625MB under the DMA throttle
                        nc.sync.dma_start(w1a[:], w1p[0:128, :])
                    xb = xs.tile([112, 4096], f8, name=f"xb_{np_}")
                    nc.sync.dma_start(xb[:], xt[ds(np_ * F + 128, 112), :])
                    xts.append((xa, xb))
                    if np_ == 0:
                        nc.sync.dma_start(w1b[:], w1p[128:240, :])

                def emit_transposes(np_):
                    for nn in range(2):
                        for q in range(4):
                            m = (2 * np_ + nn) * 4 + q
                            pt = pst.tile([128, 128], f16)
                            nc.tensor.transpose(pt[:], hT[:, ts(m, 128)],
                                                idt[:])
                            if q % 2 == 0:
                                nc.vector.tensor_copy(hsb[:, ts(m, 128)],
                                                      pt[:])
                            else:
                                nc.scalar.activation(hsb[:, ts(m, 128)],
                                                     pt[:], copyf)

                for np_ in range(4):
                    xa, xb = xts[np_]
                    for nn in range(2):
                        ncol = 2 * np_ + nn
                        p1 = ps1.tile([128, 512], f32)
                        for b in range(NB):
                            nc.tensor.matmul(p1[:], w1a[:, ts(b, 128)],
                                             xa[:, ds(b * 1024 + nn * 512,
                                                      512)],
                                             start=(b == 0), stop=False)
                        for b in range(NB):
                            nc.tensor.matmul(p1[:], w1b[:, ts(b, 128)],
                                             xb[:, ds(b * 1024 + nn * 512,
                                                      512)],
                                             start=False, stop=(b == NB - 1))
                        if nn == 0:
                            nc.scalar.activation(hT[:, ts(ncol, 512)], p1[:],
                                                 copyf)
                        else:
                            nc.vector.tensor_copy(hT[:, ts(ncol, 512)], p1[:])
                    if np_ > 0:
                        emit_transposes(np_ - 1)
                emit_transposes(3)

            # stream centered-fp8 a tiles; late consts slot in just before
            # their first use (b1t for the bias, w2s for chunk-0's head)
            if has_b1:
                nc.sync.dma_start(b1t[:], b1s[:])
            # first a tile in quarters so chunk-0's matmuls never wait
            for q4 in range(4):
                nc.sync.dma_start(at4[0][:, ds(q4 * 2048, 2048)],
                                  atp[ds(0, 128), ds(q4 * 2048, 2048)])
            nc.sync.dma_start(w2s[:], w2k[:])
            if has_b2:
                nc.sync.dma_start(b2t[:], b2r[:])
            for k in range(1, 4):
                nc.sync.dma_start(at4[k][:], atp[ds(k * 128, 128), :])

            # bias = 0.5 * rowsum(hT) + b1  (rank-1 centering correction)
            nc.vector.tensor_reduce(csum[:], hT[:], axis=AX, op=add)
            if has_b1:
                nc.vector.tensor_scalar(bc[:], csum[:], 0.5, b1t[:],
                                        op0=mult, op1=add)
            else:
                nc.vector.tensor_scalar_mul(bc[:], csum[:], 0.5)

            # phase 2+3, four chunks of 256 output rows each. Chunk c's
            # W2 head is emitted after chunk c+1's kt-matmuls so the PE
            # never waits on the ReLU ACT at a chunk boundary.
            with tc.tile_pool(name="rs", bufs=3) as rs, \
                 tc.tile_pool(name="os", bufs=4) as osb, \
                 tc.tile_pool(name="ps2", bufs=2, space="PSUM") as ps2, \
                 tc.tile_pool(name="ps3", bufs=4, space="PSUM") as ps3:
                drains = [nc.vector, nc.scalar]
                state = {"dri": 0}
                ract = [None] * 4

                def emit_head(c):
                    # w2s[32b+h, hf*480 + b*120 + li] = W2[h, hf*120 + li]
                    r = ract[c]
                    for ss in range(2):
                        o = osb.tile([128, 960], f16)
                        for hf in range(2):
                            p3 = ps3.tile([128, 480], f32)
                            if has_b2:
                                nc.tensor.matmul(
                                    p3[:], ones[:], b2t[:, ts(hf, 480)],
                                    start=True, stop=False)
                            nc.tensor.matmul(
                                p3[:], r[:, ts(ss, 128)],
                                w2s[:, ts(hf, 480)],
                                start=(not has_b2), stop=True)
                            dst = o[:, ds(hf * 480, 480)]
                            eng = drains[state["dri"] % 2]
                            state["dri"] += 1
                            if eng is nc.scalar:
                                nc.scalar.activation(dst, p3[:], copyf)
                            else:
                                eng.tensor_copy(dst, p3[:])
                        nc.sync.dma_start(outp[:, ts(c * 2 + ss, 960)], o[:])

                for c in range(4):
                    pa = ps2.tile([128, 256], f32)
                    for kt in range(32):
                        nc.tensor.matmul(
                            pa[:], hsb[:, ts(kt, 128)],
                            at4[c][:, ds(kt * 256, 256)],
                            start=(kt == 0), stop=(kt == 31))
                        if c == 3 and kt == 15:
                            emit_head(2)
                    r = rs.tile([128, 256], f16)
                    nc.scalar.activation(r[:], pa[:], relu, bias=bc[:])
                    ract[c] = r
                    if 1 <= c <= 2:
                        emit_head(c - 1)
                emit_head(3)

    nc.compile()
    return nc


def kernel(x, a, W1, b1, W2, b2):
    global last_exec_time_ns, last_profile_json
    from concourse.bass_utils import run_bass_kernel_spmd

    x = np.asarray(x, np.float32)
    a = np.asarray(a, np.float32)
    W1 = np.asarray(W1, np.float32)
    b1 = np.asarray(b1, np.float32)
    W2 = np.asarray(W2, np.float32)
    b2 = np.asarray(b2, np.float32)

    has_b1 = bool(np.any(b1))
    has_b2 = bool(np.any(b2))
    key = ("nc", has_b1, has_b2)
    if key not in _cache:
        _cache[key] = _build(has_b1, has_b2)
    nc = _cache[key]

    # xt[g]: [960, 4096] f16; xt[np*240+f, b*1024+nn*512+q] =
    #   x[4g+b, (2np+nn)*512+q, f]
    xg = []
    for g in range(2):
        arr = x[g * NB:(g + 1) * NB]            # [4, 4096, 240]
        arr = arr.transpose(2, 0, 1)            # [f, b, n]
        arr = arr.reshape(F, NB, 4, 2, 512)     # [f, b, np, nn, q]
        arr = arr.transpose(2, 0, 1, 3, 4)      # [np, f, b, nn, q]
        xg.append(np.ascontiguousarray(
            arr.reshape(4 * F, N)).astype(ml_dtypes.float8_e3m4))

    # atp[j]: [512, 8192] f8e3 chunk-major; atp[c*128+p, kt*256+q] =
    #   e3m4(a[j*1024 + c*256 + q, kt*128+p] - 0.5)
    a8 = (a.T - np.float32(0.5)).astype(ml_dtypes.float8_e3m4)  # [m, n_out]
    aj = []
    for j in range(4):
        Aj = a8[:, j * NRC:(j + 1) * NRC]       # [4096 m, 1024 n]
        arr = Aj.reshape(32, 128, 4, 256)       # [kt, p, c, q]
        arr = arr.transpose(2, 1, 0, 3)         # [c, p, kt, q]
        aj.append(np.ascontiguousarray(arr.reshape(512, 8192)))

    w1p = np.zeros((F, 512), np.float16)
    for b in range(NB):
        w1p[:, 128 * b + 32 * b:128 * b + 32 * b + 32] = W1.astype(np.float16)
    # w2k[32b+h, hf*480 + b*120 + li] = W2[h, hf*120 + li]; zeros elsewhere
    w2k = np.zeros((128, 960), np.float16)
    for hf in range(2):
        for b in range(NB):
            w2k[32 * b:32 * b + 32, 480 * hf + 120 * b:480 * hf + 120 * b + 120] = \
                W2[:, 120 * hf:120 * hf + 120].astype(np.float16)
    b1s = np.ascontiguousarray(np.tile(b1, 4).reshape(128, 1))

    ins = []
    for c in range(8):
        g, j = c // 4, c % 4
        d = {"xt": xg[g], "atp": aj[j], "w1p": w1p, "w2k": w2k}
        if has_b1:
            d["b1s"] = b1s
        if has_b2:
            b2r = np.empty((1, 960), np.float16)
            for hf in range(2):
                for b in range(NB):
                    b2r[0, 480 * hf + 120 * b:480 * hf + 120 * b + 120] = \
                        b2[120 * hf:120 * hf + 120].astype(np.float16)
            d["b2r"] = b2r
        ins.append(d)

    trace = TRACE
    if trace:
        try:
            _install_ntff_hook()
        except Exception:
            trace = False
    r = run_bass_kernel_spmd(nc, ins, list(range(8)), trace=trace)
    last_exec_time_ns = r.exec_time_ns
    last_profile_json = r.profile_json

    res = np.empty((B, N, L), np.float32)
    for c in range(8):
        g, j = c // 4, c % 4
        # outp[p, S, hf, b, li]; n = S*128 + p; l = hf*120+li
        arr = r.results[c]["outp"].reshape(128, 8, 2, NB, 120)
        res[g * NB:(g + 1) * NB, j * NRC:(j + 1) * NRC, :] = \
            arr.transpose(3, 1, 0, 2, 4).reshape(NB, NRC, L).astype(np.float32)
    return res
